# revision 1
# baseline (speedup 1.0000x reference)
"""Point-Transformer attention block on 8 Trainium2 NeuronCores.

Shards the points axis N across 8 cores (all ops are pointwise in N).
Per core: channels live on SBUF partitions, pixels (n, k) on the free dim.
All 1x1 convs are TensorE matmuls with weights stationary.

Fusions vs the reference:
  - x2/x3 in one matmul: stationary [w3; -w2] -> psum holds [x3_pre | -x2_pre]
  - ptf = pw2 @ relu(pw1 @ ppfs) accumulated into the same psum via [pw2; pw2]
    so psum = [x3_pre + ptf | d] with d = ptf - x2_pre
  - x1 / xfs / cw1 folded:  h = relu( sum_k cw1_k^T @ d_k
                                      + (cw1s @ w1) @ x + cw1s @ (b1 - b2) )
  - softmax: exp -> group-sum via 0/1 matmul -> reciprocal; normalization is
    applied after the weighted k-sum (softmax denominator commutes with sum_k)
  - identity skip-connection added via identity-matrix matmul accumulation
"""

import numpy as np

B, CIN, N, K = 4, 64, 16384, 16
MID, OUT, PT, SHARE = 64, 128, 8, 8
G = MID // SHARE  # 8 softmax groups
NCORES = 8
NS = N // NCORES          # points per core per batch
TP = 64                   # points per tile
TPK = TP * K              # pixels per tile (1024)
FD = 512                  # matmul free-dim chunk (one PSUM bank)
NCH = TPK // FD           # pixel chunks per tile (2)
KPC = K // NCH            # k-slices per chunk (8)


def _build_consts(w0, b0, w1, b1, w2, b2, w3, b3, pw1, pw2, cw1, cw2, cb2,
                  wout, bout):
    f32 = np.float32
    c = {}
    c["w0t"] = np.ascontiguousarray(w0.T, f32)                      # [64,128]
    c["w23t"] = np.ascontiguousarray(
        np.concatenate([w3, -w2], axis=0).T, f32)                   # [128,128]
    c["pw2t2"] = np.ascontiguousarray(
        np.concatenate([pw2, pw2], axis=0).T, f32)                  # [64,128]
    c["pw1t"] = np.ascontiguousarray(pw1.T, f32)                    # [8,64]
    cw1r = cw1.reshape(G, MID, K)
    cw1s = cw1r.sum(-1)                                             # [8,64]
    c["cat"] = np.ascontiguousarray((cw1s @ w1).T, f32)             # [128,8]
    c["gkt"] = np.ascontiguousarray(
        np.concatenate([cw1r[:, :, k].T for k in range(K)], axis=1), f32)  # [64,128]
    c["hb"] = np.ascontiguousarray((cw1s @ (b1 - b2))[:, None], f32)  # [8,1]
    c["cw2t"] = np.ascontiguousarray(cw2.T, f32)                    # [8,128]
    c["cb2"] = np.ascontiguousarray(cb2[:, None], f32)              # [128,1]
    bsum = np.zeros((OUT, G), f32)
    for g in range(G):
        bsum[g * K:(g + 1) * K, g] = 1.0
    c["bsum"] = bsum                                                # [128,8]
    bksel = np.zeros((OUT, K * MID), f32)
    for k in range(K):
        for m in range(MID):
            bksel[(m % G) * K + k, k * MID + m] = 1.0
    c["bksel"] = bksel                                              # [128,1024]
    bsum64 = np.zeros((G, MID), f32)
    for m in range(MID):
        bsum64[m % G, m] = 1.0
    c["bsum64"] = bsum64                                            # [8,64]
    c["woutt"] = np.ascontiguousarray(wout.T, f32)                  # [64,128]
    c["eye"] = np.eye(OUT, dtype=f32)                               # [128,128]
    c["b0"] = np.ascontiguousarray(b0[:, None], f32)                # [128,1]
    c["b3"] = np.ascontiguousarray(b3[:, None], f32)                # [64,1]
    c["bout"] = np.ascontiguousarray(bout[:, None], f32)            # [128,1]
    return c


def _build_program(ns_pts):
    import concourse.bass as bass
    import concourse.tile as tile
    from concourse import mybir
    from contextlib import ExitStack

    f32 = mybir.dt.float32
    AF = mybir.ActivationFunctionType
    nt = ns_pts // TP

    nc = bass.Bass()
    feats_d = nc.declare_dram_parameter("feats", [B, CIN, ns_pts * K], f32, isOutput=False)
    ppfs_d = nc.declare_dram_parameter("ppfs", [B, PT, ns_pts * K], f32, isOutput=False)
    cshape = dict(
        w0t=[CIN, OUT], w23t=[OUT, OUT], pw2t2=[MID, OUT], pw1t=[PT, MID],
        cat=[OUT, G], gkt=[MID, K * G], hb=[G, 1], cw2t=[G, OUT], cb2=[OUT, 1],
        bsum=[OUT, G], bksel=[OUT, K * MID], bsum64=[G, MID], woutt=[MID, OUT],
        eye=[OUT, OUT], b0=[OUT, 1], b3=[MID, 1], bout=[OUT, 1],
    )
    cdram = {k: nc.declare_dram_parameter(k, v, f32, isOutput=False)
             for k, v in cshape.items()}
    out_d = nc.declare_dram_parameter("out", [B, OUT, ns_pts], f32, isOutput=True)

    with tile.TileContext(nc) as tc, ExitStack() as ctx:
        consts = ctx.enter_context(tc.tile_pool(name="consts", bufs=1))
        ct = {k: consts.tile_from(v[:], name=k) for k, v in cdram.items()}

        io = ctx.enter_context(tc.tile_pool(name="io", bufs=3))
        sb = ctx.enter_context(tc.tile_pool(name="sb", bufs=2))
        pt_pool = ctx.enter_context(tc.tile_pool(name="pt", bufs=3))
        ps_pix = ctx.enter_context(tc.tile_pool(name="ps_pix", bufs=3, space="PSUM"))
        ps_sm = ctx.enter_context(tc.tile_pool(name="ps_sm", bufs=2, space="PSUM"))

        for b in range(B):
            for t in range(nt):
                pk = slice(t * TPK, (t + 1) * TPK)
                pn = slice(t * TP, (t + 1) * TP)

                ft = io.tile([CIN, TPK], f32, tag="ft")
                nc.sync.dma_start(ft[:], feats_d[b, :, pk])
                pf = io.tile([PT, TPK], f32, tag="pf")
                nc.sync.dma_start(pf[:], ppfs_d[b, :, pk])

                # xn = relu(w0 @ feats + b0)            [128, TPK]
                xn = sb.tile([OUT, TPK], f32, tag="xn")
                ps = ps_pix.tile([OUT, TPK], f32, tag="ps")
                for cc in range(NCH):
                    cs = slice(cc * FD, (cc + 1) * FD)
                    nc.tensor.matmul(out=ps[:, cs], lhsT=ct["w0t"][:],
                                     rhs=ft[:, cs], start=True, stop=True)
                nc.scalar.activation(xn[:], ps[:], AF.Relu, bias=ct["b0"][:])

                # r = relu(pw1 @ ppfs)                  [64, TPK]
                r = sb.tile([MID, TPK], f32, tag="r")
                ps = ps_pix.tile([OUT, TPK], f32, tag="ps")
                for cc in range(NCH):
                    cs = slice(cc * FD, (cc + 1) * FD)
                    nc.tensor.matmul(out=ps[0:MID, cs], lhsT=ct["pw1t"][:],
                                     rhs=pf[:, cs], start=True, stop=True)
                nc.scalar.activation(r[:], ps[0:MID, :], AF.Relu)

                # psum = [w3@xn + pw2@r | -w2@xn + pw2@r] -> x3 (+b3), d
                x3 = sb.tile([MID, TPK], f32, tag="x3")
                dd = sb.tile([MID, TPK], f32, tag="dd")
                ps = ps_pix.tile([OUT, TPK], f32, tag="ps")
                for cc in range(NCH):
                    cs = slice(cc * FD, (cc + 1) * FD)
                    nc.tensor.matmul(out=ps[:, cs], lhsT=ct["w23t"][:],
                                     rhs=xn[:, cs], start=True, stop=False)
                    nc.tensor.matmul(out=ps[:, cs], lhsT=ct["pw2t2"][:],
                                     rhs=r[:, cs], start=False, stop=True)
                nc.scalar.activation(x3[:], ps[0:MID, :], AF.Identity,
                                     bias=ct["b3"][:])
                nc.vector.tensor_copy(dd[:], ps[MID:OUT, :])

                xv = xn[:].rearrange("p (n k) -> p k n", k=K)[:, 0, :]  # [128,TP]
                ddk = dd[:].rearrange("p (n k) -> p k n", k=K)          # [64,16,TP]

                # h = relu(sum_k cw1_k^T @ d_k + Ca @ x + hb)   [8, TP]
                hps = ps_sm.tile([G, TP], f32, tag="pss")
                for k in range(K):
                    nc.tensor.matmul(out=hps[:], lhsT=ct["gkt"][:, k * G:(k + 1) * G],
                                     rhs=ddk[:, k, :], start=(k == 0), stop=False)
                nc.tensor.matmul(out=hps[:], lhsT=ct["cat"][:], rhs=xv,
                                 start=False, stop=True)
                h = pt_pool.tile([G, TP], f32, tag="h")
                nc.scalar.activation(h[:], hps[:], AF.Relu, bias=ct["hb"][:])

                # e = exp(cw2 @ h + cb2)                [128, TP]
                wlps = ps_sm.tile([OUT, TP], f32, tag="pss")
                nc.tensor.matmul(out=wlps[:], lhsT=ct["cw2t"][:], rhs=h[:],
                                 start=True, stop=True)
                e = pt_pool.tile([OUT, TP], f32, tag="e")
                nc.scalar.activation(e[:], wlps[:], AF.Exp, bias=ct["cb2"][:])

                # rs = 1 / group-sum(e)                 [8, TP]
                sps = ps_sm.tile([G, TP], f32, tag="pss")
                nc.tensor.matmul(out=sps[:], lhsT=ct["bsum"][:], rhs=e[:],
                                 start=True, stop=True)
                rs = pt_pool.tile([G, TP], f32, tag="rs")
                nc.vector.reciprocal(rs[:], sps[:])

                # q[m, k, n] = e[(m%8)*16+k, n] * x3[m, n, k]
                x3k = x3[:].rearrange("p (n k) -> p k n", k=K)  # [64,16,TP]
                q = sb.tile([MID, TPK], f32, tag="q")
                wf = ps_pix.tile([OUT, TPK], f32, tag="ps")
                for k in range(K):
                    nc.tensor.matmul(
                        out=wf[0:MID, k * TP:(k + 1) * TP],
                        lhsT=ct["bksel"][:, k * MID:(k + 1) * MID],
                        rhs=e[:], start=(k % KPC == 0), stop=(k % KPC == KPC - 1))
                wfv = wf[0:MID, :].rearrange("p (k n) -> p k n", k=K)
                qv = q[:].rearrange("p (k n) -> p k n", k=K)
                nc.vector.tensor_mul(qv[:], wfv[:], x3k[:])

                # U = sum_k q ; unnormalized            [64, TP]
                U = pt_pool.tile([MID, TP], f32, tag="U")
                qr = q[:].rearrange("p (k n) -> p n k", k=K)
                nc.vector.tensor_reduce(U[:], qr, axis=mybir.AxisListType.X,
                                        op=mybir.AluOpType.add)

                # o = relu(U * bcast(rs))               [64, TP]
                rbps = ps_sm.tile([MID, TP], f32, tag="pss")
                nc.tensor.matmul(out=rbps[:], lhsT=ct["bsum64"][:], rhs=rs[:],
                                 start=True, stop=True)
                vt = pt_pool.tile([MID, TP], f32, tag="vt")
                nc.vector.tensor_mul(vt[:], U[:], rbps[:])
                o = pt_pool.tile([MID, TP], f32, tag="o")
                nc.scalar.activation(o[:], vt[:], AF.Relu)

                # out = wout @ o + bout + x             [128, TP]
                ops_ = ps_sm.tile([OUT, TP], f32, tag="pss")
                nc.tensor.matmul(out=ops_[:], lhsT=ct["woutt"][:], rhs=o[:],
                                 start=True, stop=False)
                nc.tensor.matmul(out=ops_[:], lhsT=ct["eye"][:], rhs=xv,
                                 start=False, stop=True)
                res = pt_pool.tile([OUT, TP], f32, tag="res")
                nc.scalar.activation(res[:], ops_[:], AF.Identity,
                                     bias=ct["bout"][:])
                nc.sync.dma_start(out_d[b, :, pn], res[:])

    return nc


def _legalize_waits(nc):
    """This toolchain's walrus rejects >1 sync-wait per instruction; hoist
    extra waits onto same-engine event-semaphore instructions just before."""
    from concourse import mybir

    n_split = 0
    for fn in nc.m.functions:
        for bb in fn.blocks:
            insts = bb.instructions
            new_list = []
            for inst in insts:
                si = inst.sync_info
                if si is not None and si.on_wait is not None and len(si.on_wait) > 1:
                    waits = list(si.on_wait)
                    for j, w in enumerate(waits[:-1]):
                        ev = mybir.InstEventSemaphore(
                            name=f"{inst.name}-lw{j}", ins=[], outs=[])
                        ev.engine = inst.engine
                        ev.sync_info = mybir.SyncInfo(on_wait=[w], on_update=[])
                        new_list.append(ev)
                        n_split += 1
                    inst.sync_info = mybir.SyncInfo(
                        on_wait=[waits[-1]], on_update=list(si.on_update))
                new_list.append(inst)
            if len(new_list) != len(insts):
                insts[:] = new_list
    return n_split


LAST_RESULTS = None


def kernel(sm_feats, sm_ppfs, w0, b0, w1, b1, w2, b2, w3, b3,
           pw1, pw2, cw1, cw2, cb2, wout, bout):
    global LAST_RESULTS
    from concourse.bass_utils import run_bass_kernel_spmd

    consts = _build_consts(w0, b0, w1, b1, w2, b2, w3, b3, pw1, pw2,
                           cw1, cw2, cb2, wout, bout)
    nc = _build_program(NS)
    _legalize_waits(nc)

    in_maps = []
    for i in range(NCORES):
        sl = slice(i * NS, (i + 1) * NS)
        m = dict(consts)
        m["feats"] = np.ascontiguousarray(
            sm_feats[:, :, sl, :], np.float32).reshape(B, CIN, NS * K)
        m["ppfs"] = np.ascontiguousarray(
            sm_ppfs[:, :, sl, :], np.float32).reshape(B, PT, NS * K)
        in_maps.append(m)

    res = run_bass_kernel_spmd(nc, in_maps, list(range(NCORES)))
    LAST_RESULTS = res
    shards = [res.results[i]["out"] for i in range(NCORES)]
    return np.concatenate(shards, axis=2)



# revision 5
# speedup vs baseline: 2.9666x; 2.9666x over previous
"""Point-Transformer attention block on 8 Trainium2 NeuronCores.

Shards the points axis N across 8 cores (all ops are pointwise in N).
Per core: channels on SBUF partitions, pixels (n, k) on the free dim.
All 1x1 convs are TensorE matmuls with bf16 operands (1 cycle/row at any
moving size, vs 4 for fp32) accumulating in fp32 PSUM.

Fusions vs the reference:
  - x3/d in one psum: stationary [w3; -w2] + [pw2; pw2] -> psum rows
    0:64 = x3_pre (+ptf), 64:128 = d = ptf - x2_pre; one scalar-engine
    activation adds [b3; 0] and materializes both halves as bf16.
  - ppfs host-packed [16, npix/2] (chunk pairs stacked on partitions) so
    one blockdiag(pw1) matmul computes two chunks of r at once; r lives
    [128, npix/2] and the pw2 stationaries are zero-padded to match.
  - x1 / xfs / cw1 folded:  h = relu( sum_k cw1_k^T @ d_k
                                      + (cw1s @ w1) @ x + cw1s @ (b1 - b2) )
  - softmax: exp -> group-sum via 0/1 matmul -> reciprocal -> broadcast
    back to 128 rows via 0/1 matmul -> normalize e before the k-sum
  - identity skip added via identity-matrix matmul accumulation
"""

import numpy as np

B, CIN, N, K = 4, 64, 16384, 16
MID, OUT, PT, SHARE = 64, 128, 8, 8
G = MID // SHARE  # 8 softmax groups
NCORES = 8
NS = N // NCORES          # points per core per batch (2048)
P = 256                   # points per tile
PK = P * K                # pixels per tile (4096)
FD = 512                  # pixel chunk (one matmul's moving width)
NCH = PK // FD            # chunks per tile (8)


def _build_consts(w0, b0, w1, b1, w2, b2, w3, b3, pw1, pw2, cw1, cw2, cb2,
                  wout, bout):
    import ml_dtypes
    f32 = np.float32
    bf = ml_dtypes.bfloat16
    c = {}
    c["w0t"] = np.ascontiguousarray(w0.T).astype(bf)                # [64,128]
    c["w23t"] = np.ascontiguousarray(
        np.concatenate([w3, -w2], axis=0).T).astype(bf)             # [128,128]
    pw2t2 = np.ascontiguousarray(np.concatenate([pw2, pw2], axis=0).T)  # [64,128]
    z = np.zeros_like(pw2t2)
    c["pw2lo"] = np.concatenate([pw2t2, z], axis=0).astype(bf)      # [128,128]
    c["pw2hi"] = np.concatenate([z, pw2t2], axis=0).astype(bf)      # [128,128]
    pw1t = np.ascontiguousarray(pw1.T)                              # [8,64]
    pw1t2 = np.zeros((2 * PT, OUT), np.float32)
    pw1t2[0:PT, 0:MID] = pw1t
    pw1t2[PT:2 * PT, MID:OUT] = pw1t
    c["pw1t2"] = pw1t2.astype(bf)                                   # [16,128]
    cw1r = cw1.reshape(G, MID, K)
    cw1s = cw1r.sum(-1)                                             # [8,64]
    c["cat"] = np.ascontiguousarray((cw1s @ w1).T).astype(bf)       # [128,8]
    gkt = np.concatenate([cw1r[:, :, k].T for k in range(K)], axis=1)  # [64,128]
    c["gkt"] = np.concatenate(
        [np.zeros_like(gkt), gkt], axis=0).astype(bf)               # [128,128]
    c["hb"] = np.ascontiguousarray((cw1s @ (b1 - b2))[:, None], f32)  # [8,1]
    c["cw2t"] = np.ascontiguousarray(cw2.T).astype(bf)              # [8,128]
    c["cb2"] = np.ascontiguousarray(cb2[:, None], f32)              # [128,1]
    bsum = np.zeros((OUT, G), f32)
    for g in range(G):
        bsum[g * K:(g + 1) * K, g] = 1.0
    c["bsum"] = bsum.astype(bf)                                     # [128,8]
    c["bsum128"] = np.ascontiguousarray(bsum.T).astype(bf)          # [8,128]
    bksel = np.zeros((OUT, K * MID), f32)
    for k in range(K):
        for m in range(MID):
            bksel[(m % G) * K + k, k * MID + m] = 1.0
    c["bksel"] = bksel.astype(bf)                                   # [128,1024]
    c["woutt"] = np.ascontiguousarray(wout.T).astype(bf)            # [64,128]
    c["eye"] = np.eye(OUT, dtype=f32).astype(bf)                    # [128,128]
    c["b0"] = np.ascontiguousarray(b0[:, None], f32)                # [128,1]
    c["b3z"] = np.concatenate(
        [b3, np.zeros(MID, f32)])[:, None].astype(f32)              # [128,1]
    c["bout"] = np.ascontiguousarray(bout[:, None], f32)            # [128,1]
    return c


def _build_program(ns_pts):
    import concourse.bass as bass
    import concourse.tile as tile
    from concourse import mybir
    from contextlib import ExitStack

    f32 = mybir.dt.float32
    bf16 = mybir.dt.bfloat16
    AF = mybir.ActivationFunctionType
    ALU = mybir.AluOpType
    AX = mybir.AxisListType
    nt = ns_pts // P

    nc = bass.Bass()
    feats_d = nc.declare_dram_parameter("feats", [B, CIN, ns_pts * K], bf16, isOutput=False)
    ppfs_d = nc.declare_dram_parameter("ppfs", [B, 2 * PT, ns_pts * K // 2], bf16, isOutput=False)
    cshape = dict(
        w0t=([CIN, OUT], bf16), w23t=([OUT, OUT], bf16),
        pw2lo=([OUT, OUT], bf16), pw2hi=([OUT, OUT], bf16),
        pw1t2=([2 * PT, OUT], bf16), cat=([OUT, G], bf16), gkt=([OUT, K * G], bf16),
        hb=([G, 1], f32), cw2t=([G, OUT], bf16), cb2=([OUT, 1], f32),
        bsum=([OUT, G], bf16), bsum128=([G, OUT], bf16), bksel=([OUT, K * MID], bf16),
        woutt=([MID, OUT], bf16), eye=([OUT, OUT], bf16),
        b0=([OUT, 1], f32), b3z=([OUT, 1], f32), bout=([OUT, 1], f32),
    )
    cdram = {k: nc.declare_dram_parameter(k, v[0], v[1], isOutput=False)
             for k, v in cshape.items()}
    out_d = nc.declare_dram_parameter("out", [B, OUT, ns_pts], f32, isOutput=True)

    with tile.TileContext(nc) as tc, ExitStack() as ctx:
        consts = ctx.enter_context(tc.tile_pool(name="consts", bufs=1))
        ct = {k: consts.tile_from(v[:], name=k) for k, v in cdram.items()}

        io = ctx.enter_context(tc.tile_pool(name="io", bufs=2))
        sb = ctx.enter_context(tc.tile_pool(name="sb", bufs=2))
        small = ctx.enter_context(tc.tile_pool(name="small", bufs=2))
        pix = ctx.enter_context(tc.tile_pool(name="pix", bufs=2, space="PSUM"))
        ptp = ctx.enter_context(tc.tile_pool(name="ptp", bufs=2, space="PSUM"))
        wfp = ctx.enter_context(tc.tile_pool(name="wfp", bufs=2, space="PSUM"))

        for b in range(B):
            for t in range(nt):
                px = slice(t * PK, (t + 1) * PK)
                px2 = slice(t * PK // 2, (t + 1) * PK // 2)
                pn = slice(t * P, (t + 1) * P)

                ft = io.tile([CIN, PK], bf16, tag="ft")
                nc.sync.dma_start(ft[:], feats_d[b, :, px])
                pf = io.tile([2 * PT, PK // 2], bf16, tag="pf")
                nc.sync.dma_start(pf[:], ppfs_d[b, :, px2])

                # xn = relu(w0 @ feats + b0)            [128, PK] bf16
                xn = sb.tile([OUT, PK], bf16, tag="xn")
                for u in range(NCH // 2):
                    ps = pix.tile([OUT, 2 * FD], f32, tag="pix")
                    for h in range(2):
                        cs = slice((2 * u + h) * FD, (2 * u + h + 1) * FD)
                        nc.tensor.matmul(out=ps[:, h * FD:(h + 1) * FD],
                                         lhsT=ct["w0t"][:], rhs=ft[:, cs],
                                         start=True, stop=True)
                    nc.scalar.activation(xn[:, 2 * u * FD:(2 * u + 2) * FD],
                                         ps[:], AF.Relu, bias=ct["b0"][:])

                # r = relu(pw1 @ ppfs), chunk c on rows 64*(c%2),
                # cols (c//2)*FD                        [128, PK/2] bf16
                r = sb.tile([OUT, PK // 2], bf16, tag="r")
                for u in range(NCH // 4):
                    ps = pix.tile([OUT, 2 * FD], f32, tag="pix")
                    for h in range(2):
                        js = slice((2 * u + h) * FD, (2 * u + h + 1) * FD)
                        nc.tensor.matmul(out=ps[:, h * FD:(h + 1) * FD],
                                         lhsT=ct["pw1t2"][:], rhs=pf[:, js],
                                         start=True, stop=True)
                    nc.scalar.activation(r[:, 2 * u * FD:(2 * u + 2) * FD],
                                         ps[:], AF.Relu)

                # psum = [w3@xn + pw2@r | -w2@xn + pw2@r]; +[b3|0] -> xd bf16
                xd = sb.tile([OUT, PK], bf16, tag="xd")
                for u in range(NCH // 2):
                    ps = pix.tile([OUT, 2 * FD], f32, tag="pix")
                    for h in range(2):
                        c = 2 * u + h
                        cs = slice(c * FD, (c + 1) * FD)
                        rcs = slice((c // 2) * FD, (c // 2 + 1) * FD)
                        hs = slice(h * FD, (h + 1) * FD)
                        nc.tensor.matmul(out=ps[:, hs], lhsT=ct["w23t"][:],
                                         rhs=xn[:, cs], start=True, stop=False)
                        nc.tensor.matmul(out=ps[:, hs],
                                         lhsT=ct["pw2lo" if c % 2 == 0 else "pw2hi"][:],
                                         rhs=r[:, rcs], start=False, stop=True)
                    nc.scalar.activation(xd[:, 2 * u * FD:(2 * u + 2) * FD],
                                         ps[:], AF.Identity, bias=ct["b3z"][:])

                xv = xn[:].rearrange("p (n k) -> p k n", k=K)[:, 0, :]     # [128,P]
                dview = xd[:].rearrange("p (n k) -> p k n", k=K)           # [128,16,P]
                x3view = xd[0:MID, :].rearrange("p (n k) -> p k n", k=K)   # [64,16,P]

                # h = relu(sum_k cw1_k^T @ d_k + Ca @ x + hb)   [8, P]
                hps = ptp.tile([G, P], f32, tag="pt")
                for k in range(K):
                    nc.tensor.matmul(out=hps[:], lhsT=ct["gkt"][:, k * G:(k + 1) * G],
                                     rhs=dview[:, k, :], start=(k == 0), stop=False)
                nc.tensor.matmul(out=hps[:], lhsT=ct["cat"][:], rhs=xv,
                                 start=False, stop=True)
                h = small.tile([G, P], bf16, tag="h")
                nc.scalar.activation(h[:], hps[:], AF.Relu, bias=ct["hb"][:])

                # e = exp(cw2 @ h + cb2)                [128, P] bf16
                eps = ptp.tile([OUT, P], f32, tag="pt")
                nc.tensor.matmul(out=eps[:], lhsT=ct["cw2t"][:], rhs=h[:],
                                 start=True, stop=True)
                e = small.tile([OUT, P], bf16, tag="e")
                nc.scalar.activation(e[:], eps[:], AF.Exp, bias=ct["cb2"][:])

                # rs = 1 / group-sum(e);  ep = e * bcast(rs)   [128, P] bf16
                sps = ptp.tile([G, P], f32, tag="pt")
                nc.tensor.matmul(out=sps[:], lhsT=ct["bsum"][:], rhs=e[:],
                                 start=True, stop=True)
                rs = small.tile([G, P], bf16, tag="rs")
                with nc.allow_low_precision("softmax reciprocal in bf16"):
                    nc.vector.reciprocal(rs[:], sps[:])
                rsb = ptp.tile([OUT, P], f32, tag="pt")
                nc.tensor.matmul(out=rsb[:], lhsT=ct["bsum128"][:], rhs=rs[:],
                                 start=True, stop=True)
                ep = small.tile([OUT, P], bf16, tag="ep")
                nc.vector.tensor_mul(ep[:], e[:], rsb[:])

                # q[m, k*P+n] = ep[(m%8)*16+k, n] * x3[m, n, k]
                q = sb.tile([MID, PK], bf16, tag="q")
                for k in range(K):
                    wf = wfp.tile([MID, P], f32, tag="wf")
                    nc.tensor.matmul(out=wf[:],
                                     lhsT=ct["bksel"][:, k * MID:(k + 1) * MID],
                                     rhs=ep[:], start=True, stop=True)
                    nc.vector.tensor_mul(q[:, k * P:(k + 1) * P],
                                         x3view[:, k, :], wf[:])

                # U = sum_k q                          [64, P] f32
                U = small.tile([MID, P], f32, tag="U")
                qv = q[:].rearrange("p (k n) -> p n k", k=K)
                nc.vector.tensor_reduce(U[:], qv, axis=AX.X, op=ALU.add)

                # o = relu(U)                          [64, P] bf16
                o = small.tile([MID, P], bf16, tag="o")
                nc.gpsimd.tensor_scalar_max(o[:], U[:], 0.0)

                # out = wout @ o + x + bout            [128, P] f32
                ops_ = ptp.tile([OUT, P], f32, tag="pt")
                nc.tensor.matmul(out=ops_[:], lhsT=ct["woutt"][:], rhs=o[:],
                                 start=True, stop=False)
                nc.tensor.matmul(out=ops_[:], lhsT=ct["eye"][:], rhs=xv,
                                 start=False, stop=True)
                res = small.tile([OUT, P], f32, tag="res")
                nc.vector.tensor_scalar_add(res[:], ops_[:], ct["bout"][:])
                nc.sync.dma_start(out_d[b, :, pn], res[:])

    return nc


def _legalize_waits(nc):
    """This toolchain's walrus rejects >1 sync-wait per instruction; hoist
    extra waits onto same-engine event-semaphore instructions just before."""
    from concourse import mybir

    n_split = 0
    for fn in nc.m.functions:
        for bb in fn.blocks:
            insts = bb.instructions
            new_list = []
            for inst in insts:
                si = inst.sync_info
                if si is not None and si.on_wait is not None and len(si.on_wait) > 1:
                    waits = list(si.on_wait)
                    for j, w in enumerate(waits[:-1]):
                        ev = mybir.InstEventSemaphore(
                            name=f"{inst.name}-lw{j}", ins=[], outs=[])
                        ev.engine = inst.engine
                        ev.sync_info = mybir.SyncInfo(on_wait=[w], on_update=[])
                        new_list.append(ev)
                        n_split += 1
                    inst.sync_info = mybir.SyncInfo(
                        on_wait=[waits[-1]], on_update=list(si.on_update))
                new_list.append(inst)
            if len(new_list) != len(insts):
                insts[:] = new_list
    return n_split


LAST_RESULTS = None


def kernel(sm_feats, sm_ppfs, w0, b0, w1, b1, w2, b2, w3, b3,
           pw1, pw2, cw1, cw2, cb2, wout, bout):
    global LAST_RESULTS
    import ml_dtypes
    from concourse.bass_utils import run_bass_kernel_spmd

    bf = ml_dtypes.bfloat16
    consts = _build_consts(w0, b0, w1, b1, w2, b2, w3, b3, pw1, pw2,
                           cw1, cw2, cb2, wout, bout)
    nc = _build_program(NS)
    _legalize_waits(nc)

    feats_bf = np.ascontiguousarray(sm_feats).astype(bf)
    ppfs_bf = np.ascontiguousarray(sm_ppfs).astype(bf)
    npix = NS * K

    in_maps = []
    for i in range(NCORES):
        sl = slice(i * NS, (i + 1) * NS)
        m = dict(consts)
        m["feats"] = np.ascontiguousarray(
            feats_bf[:, :, sl, :]).reshape(B, CIN, npix)
        pp = np.ascontiguousarray(ppfs_bf[:, :, sl, :]).reshape(B, PT, npix)
        # pack chunk pairs on partitions: [b, h*8+ch, j*512+s] = pp[b, ch, (2j+h)*512+s]
        v = pp.reshape(B, PT, npix // 1024, 2, FD)
        m["ppfs"] = np.ascontiguousarray(
            v.transpose(0, 3, 1, 2, 4)).reshape(B, 2 * PT, npix // 2)
        in_maps.append(m)

    res = run_bass_kernel_spmd(nc, in_maps, list(range(NCORES)))
    LAST_RESULTS = res
    shards = [res.results[i]["out"] for i in range(NCORES)]
    return np.concatenate(shards, axis=2)


# revision 7
# speedup vs baseline: 4.7562x; 1.6033x over previous
"""Point-Transformer attention block on 8 Trainium2 NeuronCores.

Shards the points axis N across 8 cores (all ops are pointwise in N).
Per core: channels on SBUF partitions, pixels on the free dim.  Pixels are
host-reordered k-major within each 256-point tile, so every strided (n,k)
view becomes a contiguous slice.  All matmuls use bf16 operands (1 cycle/
row) accumulating in fp32 PSUM.

Fusions vs the reference:
  - x3/d in one psum: stationary [w3; -w2] + [pw2; pw2] -> psum rows
    0:64 = x3_pre (+ptf), 64:128 = d = ptf - x2_pre; one scalar-engine
    activation adds [b3; 0] and materializes both halves as bf16.
  - ppfs host-packed [16, npix/2] (chunk pairs stacked on partitions) so
    one blockdiag(pw1) matmul computes two chunks of r at once.
  - x1 / xfs / cw1 folded:  h = relu( sum_k cw1_k^T @ d_k
                                      + (cw1s @ w1) @ x + cw1s @ (b1 - b2) )
  - softmax: exp -> group-sum via 0/1 matmul -> reciprocal -> broadcast
    back to 128 rows via 0/1 matmul -> normalize e before the k-sum
  - k-sum of q done as 16 accumulating identity matmuls into PSUM
  - identity skip added via identity-matrix matmul accumulation
The emission is software-pipelined S1(i) S3(i-1) S2(i) S4(i-1) so the
tensor engine never drains waiting on the point-space serial chain.
"""

import numpy as np

B, CIN, N, K = 4, 64, 16384, 16
MID, OUT, PT, SHARE = 64, 128, 8, 8
G = MID // SHARE  # 8 softmax groups
NCORES = 8
NS = N // NCORES          # points per core per batch (2048)
P = 256                   # points per tile
PK = P * K                # pixels per tile (4096)
FD = 512                  # pixel chunk (one matmul's moving width)
NCH = PK // FD            # chunks per tile (8)


def _build_consts(w0, b0, w1, b1, w2, b2, w3, b3, pw1, pw2, cw1, cw2, cb2,
                  wout, bout):
    import ml_dtypes
    f32 = np.float32
    bf = ml_dtypes.bfloat16
    c = {}
    c["w0t"] = np.ascontiguousarray(w0.T).astype(bf)                # [64,128]
    c["w23t"] = np.ascontiguousarray(
        np.concatenate([w3, -w2], axis=0).T).astype(bf)             # [128,128]
    pw2t2 = np.ascontiguousarray(np.concatenate([pw2, pw2], axis=0).T)  # [64,128]
    z = np.zeros_like(pw2t2)
    c["pw2lo"] = np.concatenate([pw2t2, z], axis=0).astype(bf)      # [128,128]
    c["pw2hi"] = np.concatenate([z, pw2t2], axis=0).astype(bf)      # [128,128]
    pw1t = np.ascontiguousarray(pw1.T)                              # [8,64]
    pw1t2 = np.zeros((2 * PT, OUT), np.float32)
    pw1t2[0:PT, 0:MID] = pw1t
    pw1t2[PT:2 * PT, MID:OUT] = pw1t
    c["pw1t2"] = pw1t2.astype(bf)                                   # [16,128]
    cw1r = cw1.reshape(G, MID, K)
    cw1s = cw1r.sum(-1)                                             # [8,64]
    c["cat"] = np.ascontiguousarray((cw1s @ w1).T).astype(bf)       # [128,8]
    gkt = np.concatenate([cw1r[:, :, k].T for k in range(K)], axis=1)  # [64,128]
    c["gkt"] = np.concatenate(
        [np.zeros_like(gkt), gkt], axis=0).astype(bf)               # [128,128]
    c["hb"] = np.ascontiguousarray((cw1s @ (b1 - b2))[:, None], f32)  # [8,1]
    c["cw2t"] = np.ascontiguousarray(cw2.T).astype(bf)              # [8,128]
    c["cb2"] = np.ascontiguousarray(cb2[:, None], f32)              # [128,1]
    bsum = np.zeros((OUT, G), f32)
    for g in range(G):
        bsum[g * K:(g + 1) * K, g] = 1.0
    c["bsum"] = bsum.astype(bf)                                     # [128,8]
    c["bsum128"] = np.ascontiguousarray(bsum.T).astype(bf)          # [8,128]
    bksel = np.zeros((OUT, K * MID), f32)
    for k in range(K):
        for m in range(MID):
            bksel[(m % G) * K + k, k * MID + m] = 1.0
    c["bksel"] = bksel.astype(bf)                                   # [128,1024]
    c["woutt"] = np.ascontiguousarray(wout.T).astype(bf)            # [64,128]
    c["eye"] = np.eye(OUT, dtype=f32).astype(bf)                    # [128,128]
    c["eye64"] = np.eye(MID, dtype=f32).astype(bf)                  # [64,64]
    c["b0"] = np.ascontiguousarray(b0[:, None], f32)                # [128,1]
    c["b3z"] = np.concatenate(
        [b3, np.zeros(MID, f32)])[:, None].astype(f32)              # [128,1]
    c["bout"] = np.ascontiguousarray(bout[:, None], f32)            # [128,1]
    return c


def _build_program(ns_pts):
    import concourse.bass as bass
    import concourse.tile as tile
    from concourse import mybir
    from contextlib import ExitStack

    f32 = mybir.dt.float32
    bf16 = mybir.dt.bfloat16
    AF = mybir.ActivationFunctionType
    nt = ns_pts // P

    nc = bass.Bass()
    feats_d = nc.declare_dram_parameter("feats", [B, CIN, ns_pts * K], bf16, isOutput=False)
    ppfs_d = nc.declare_dram_parameter("ppfs", [B, 2 * PT, ns_pts * K // 2], bf16, isOutput=False)
    cshape = dict(
        w0t=([CIN, OUT], bf16), w23t=([OUT, OUT], bf16),
        pw2lo=([OUT, OUT], bf16), pw2hi=([OUT, OUT], bf16),
        pw1t2=([2 * PT, OUT], bf16), cat=([OUT, G], bf16), gkt=([OUT, K * G], bf16),
        hb=([G, 1], f32), cw2t=([G, OUT], bf16), cb2=([OUT, 1], f32),
        bsum=([OUT, G], bf16), bsum128=([G, OUT], bf16), bksel=([OUT, K * MID], bf16),
        woutt=([MID, OUT], bf16), eye=([OUT, OUT], bf16), eye64=([MID, MID], bf16),
        b0=([OUT, 1], f32), b3z=([OUT, 1], f32), bout=([OUT, 1], f32),
    )
    cdram = {k: nc.declare_dram_parameter(k, v[0], v[1], isOutput=False)
             for k, v in cshape.items()}
    out_d = nc.declare_dram_parameter("out", [B, OUT, ns_pts], f32, isOutput=True)

    ntot = B * nt

    with tile.TileContext(nc) as tc, ExitStack() as ctx:
        consts = ctx.enter_context(tc.tile_pool(name="consts", bufs=1))
        ct = {k: consts.tile_from(v[:], name=k) for k, v in cdram.items()}

        io = ctx.enter_context(tc.tile_pool(name="io", bufs=2))
        sb = ctx.enter_context(tc.tile_pool(name="sb", bufs=2))
        small = ctx.enter_context(tc.tile_pool(name="small", bufs=2))
        pix = ctx.enter_context(tc.tile_pool(name="pix", bufs=2, space="PSUM"))
        ptp = ctx.enter_context(tc.tile_pool(name="ptp", bufs=2, space="PSUM"))
        wfp = ctx.enter_context(tc.tile_pool(name="wfp", bufs=2, space="PSUM"))

        st = {}  # per-tile live handles

        def emit_dma(i):
            b, t = divmod(i, nt)
            px = slice(t * PK, (t + 1) * PK)
            px2 = slice(t * PK // 2, (t + 1) * PK // 2)
            ft = io.tile([CIN, PK], bf16, tag="ft")
            nc.sync.dma_start(ft[:], feats_d[b, :, px])
            pf = io.tile([2 * PT, PK // 2], bf16, tag="pf")
            nc.sync.dma_start(pf[:], ppfs_d[b, :, px2])
            st[i] = dict(ft=ft, pf=pf)

        def emit_s1(i):
            s = st[i]
            ft, pf = s["ft"], s["pf"]
            # xn = relu(w0 @ feats + b0)            [128, PK] bf16
            xn = sb.tile([OUT, PK], bf16, tag="xn")
            for u in range(NCH // 2):
                ps = pix.tile([OUT, 2 * FD], f32, tag="pix")
                for h in range(2):
                    cs = slice((2 * u + h) * FD, (2 * u + h + 1) * FD)
                    nc.tensor.matmul(out=ps[:, h * FD:(h + 1) * FD],
                                     lhsT=ct["w0t"][:], rhs=ft[:, cs],
                                     start=True, stop=True)
                nc.scalar.activation(xn[:, 2 * u * FD:(2 * u + 2) * FD],
                                     ps[:], AF.Relu, bias=ct["b0"][:])
            # r = relu(pw1 @ ppfs), chunk c on rows 64*(c%2), cols (c//2)*FD
            r = sb.tile([OUT, PK // 2], bf16, tag="r")
            for u in range(NCH // 4):
                ps = pix.tile([OUT, 2 * FD], f32, tag="pix")
                for h in range(2):
                    js = slice((2 * u + h) * FD, (2 * u + h + 1) * FD)
                    nc.tensor.matmul(out=ps[:, h * FD:(h + 1) * FD],
                                     lhsT=ct["pw1t2"][:], rhs=pf[:, js],
                                     start=True, stop=True)
                nc.vector.tensor_scalar_max(r[:, 2 * u * FD:(2 * u + 2) * FD],
                                            ps[:], 0.0)
            # psum = [w3@xn + pw2@r | -w2@xn + pw2@r]; +[b3|0] -> xd bf16
            xd = sb.tile([OUT, PK], bf16, tag="xd")
            for u in range(NCH // 2):
                ps = pix.tile([OUT, 2 * FD], f32, tag="pix")
                for h in range(2):
                    c = 2 * u + h
                    cs = slice(c * FD, (c + 1) * FD)
                    rcs = slice((c // 2) * FD, (c // 2 + 1) * FD)
                    hs = slice(h * FD, (h + 1) * FD)
                    nc.tensor.matmul(out=ps[:, hs], lhsT=ct["w23t"][:],
                                     rhs=xn[:, cs], start=True, stop=False)
                    nc.tensor.matmul(out=ps[:, hs],
                                     lhsT=ct["pw2lo" if c % 2 == 0 else "pw2hi"][:],
                                     rhs=r[:, rcs], start=False, stop=True)
                nc.scalar.activation(xd[:, 2 * u * FD:(2 * u + 2) * FD],
                                     ps[:], AF.Identity, bias=ct["b3z"][:])
            s["xn"], s["r"], s["xd"] = xn, r, xd

        def emit_s2(i):
            # point-space head: h, e, softmax denominators, normalized ep
            s = st[i]
            xn, xd = s["xn"], s["xd"]
            # h = relu(sum_k cw1_k^T @ d_k + Ca @ x + hb)   [8, P]
            hps = ptp.tile([G, P], f32, tag="pt")
            for k in range(K):
                nc.tensor.matmul(out=hps[:], lhsT=ct["gkt"][:, k * G:(k + 1) * G],
                                 rhs=xd[:, k * P:(k + 1) * P],
                                 start=(k == 0), stop=False)
            nc.tensor.matmul(out=hps[:], lhsT=ct["cat"][:], rhs=xn[:, 0:P],
                             start=False, stop=True)
            h = small.tile([G, P], bf16, tag="h")
            nc.scalar.activation(h[:], hps[:], AF.Relu, bias=ct["hb"][:])
            # e = exp(cw2 @ h + cb2)                [128, P] bf16
            eps = ptp.tile([OUT, P], f32, tag="pt")
            nc.tensor.matmul(out=eps[:], lhsT=ct["cw2t"][:], rhs=h[:],
                             start=True, stop=True)
            e = small.tile([OUT, P], bf16, tag="e")
            nc.scalar.activation(e[:], eps[:], AF.Exp, bias=ct["cb2"][:])
            # rs = 1 / group-sum(e);  ep = e * bcast(rs)
            sps = ptp.tile([G, P], f32, tag="pt")
            nc.tensor.matmul(out=sps[:], lhsT=ct["bsum"][:], rhs=e[:],
                             start=True, stop=True)
            rs = small.tile([G, P], bf16, tag="rs")
            with nc.allow_low_precision("softmax reciprocal in bf16"):
                nc.vector.reciprocal(rs[:], sps[:])
            rsb = ptp.tile([OUT, P], f32, tag="pt")
            nc.tensor.matmul(out=rsb[:], lhsT=ct["bsum128"][:], rhs=rs[:],
                             start=True, stop=True)
            ep = small.tile([OUT, P], bf16, tag="ep")
            nc.vector.tensor_mul(ep[:], e[:], rsb[:])
            s["ep"] = ep

        def emit_s3(i):
            # attention apply: q_k = x3_k * (bksel_k @ ep); U = sum_k q_k
            s = st[i]
            xd, ep = s["xd"], s["ep"]
            q = sb.tile([MID, PK], bf16, tag="q")
            for j in range(K // 2):
                wf = wfp.tile([MID, 2 * P], f32, tag="wf")
                for h in range(2):
                    k = 2 * j + h
                    nc.tensor.matmul(out=wf[:, h * P:(h + 1) * P],
                                     lhsT=ct["bksel"][:, k * MID:(k + 1) * MID],
                                     rhs=ep[:], start=True, stop=True)
                nc.vector.tensor_mul(q[:, 2 * j * P:(2 * j + 2) * P],
                                     xd[0:MID, 2 * j * P:(2 * j + 2) * P], wf[:])
            ups = ptp.tile([MID, P], f32, tag="pt")
            for k in range(K):
                nc.tensor.matmul(out=ups[:], lhsT=ct["eye64"][:],
                                 rhs=q[:, k * P:(k + 1) * P],
                                 start=(k == 0), stop=(k == K - 1))
            o = small.tile([MID, P], bf16, tag="o")
            nc.scalar.activation(o[:], ups[:], AF.Relu)
            s["o"] = o

        def emit_s4(i):
            # out = wout @ o + x + bout            [128, P] f32
            b, t = divmod(i, nt)
            pn = slice(t * P, (t + 1) * P)
            s = st[i]
            xn, o = s["xn"], s["o"]
            ops_ = ptp.tile([OUT, P], f32, tag="pt")
            nc.tensor.matmul(out=ops_[:], lhsT=ct["woutt"][:], rhs=o[:],
                             start=True, stop=False)
            nc.tensor.matmul(out=ops_[:], lhsT=ct["eye"][:], rhs=xn[:, 0:P],
                             start=False, stop=True)
            res = small.tile([OUT, P], f32, tag="res")
            nc.vector.tensor_scalar_add(res[:], ops_[:], ct["bout"][:])
            nc.sync.dma_start(out_d[b, :, pn], res[:])
            del st[i]

        emit_dma(0)
        for i in range(ntot):
            if i + 1 < ntot:
                emit_dma(i + 1)
            emit_s1(i)
            if i >= 1:
                emit_s3(i - 1)
            emit_s2(i)
            if i >= 1:
                emit_s4(i - 1)
        emit_s3(ntot - 1)
        emit_s4(ntot - 1)

    return nc


def _legalize_waits(nc):
    """This toolchain's walrus rejects >1 sync-wait per instruction; hoist
    extra waits onto same-engine event-semaphore instructions just before."""
    from concourse import mybir

    n_split = 0
    for fn in nc.m.functions:
        for bb in fn.blocks:
            insts = bb.instructions
            new_list = []
            for inst in insts:
                si = inst.sync_info
                if si is not None and si.on_wait is not None and len(si.on_wait) > 1:
                    waits = list(si.on_wait)
                    for j, w in enumerate(waits[:-1]):
                        ev = mybir.InstEventSemaphore(
                            name=f"{inst.name}-lw{j}", ins=[], outs=[])
                        ev.engine = inst.engine
                        ev.sync_info = mybir.SyncInfo(on_wait=[w], on_update=[])
                        new_list.append(ev)
                        n_split += 1
                    inst.sync_info = mybir.SyncInfo(
                        on_wait=[waits[-1]], on_update=list(si.on_update))
                new_list.append(inst)
            if len(new_list) != len(insts):
                insts[:] = new_list
    return n_split


LAST_RESULTS = None


def kernel(sm_feats, sm_ppfs, w0, b0, w1, b1, w2, b2, w3, b3,
           pw1, pw2, cw1, cw2, cb2, wout, bout):
    global LAST_RESULTS
    import ml_dtypes
    from concourse.bass_utils import run_bass_kernel_spmd

    bf = ml_dtypes.bfloat16
    consts = _build_consts(w0, b0, w1, b1, w2, b2, w3, b3, pw1, pw2,
                           cw1, cw2, cb2, wout, bout)
    nc = _build_program(NS)
    _legalize_waits(nc)

    feats_bf = np.ascontiguousarray(sm_feats).astype(bf)
    ppfs_bf = np.ascontiguousarray(sm_ppfs).astype(bf)
    npix = NS * K

    in_maps = []
    for i in range(NCORES):
        sl = slice(i * NS, (i + 1) * NS)
        m = dict(consts)
        # k-major pixel order within each 256-point tile
        f = feats_bf[:, :, sl, :].reshape(B, CIN, NS // P, P, K)
        m["feats"] = np.ascontiguousarray(
            f.transpose(0, 1, 2, 4, 3)).reshape(B, CIN, npix)
        p = ppfs_bf[:, :, sl, :].reshape(B, PT, NS // P, P, K)
        pp = np.ascontiguousarray(
            p.transpose(0, 1, 2, 4, 3)).reshape(B, PT, npix)
        # pack chunk pairs on partitions: [b, h*8+ch, j*512+s] = pp[b, ch, (2j+h)*512+s]
        v = pp.reshape(B, PT, npix // 1024, 2, FD)
        m["ppfs"] = np.ascontiguousarray(
            v.transpose(0, 3, 1, 2, 4)).reshape(B, 2 * PT, npix // 2)
        in_maps.append(m)

    res = run_bass_kernel_spmd(nc, in_maps, list(range(NCORES)))
    LAST_RESULTS = res
    shards = [res.results[i]["out"] for i in range(NCORES)]
    return np.concatenate(shards, axis=2)


# revision 11
# speedup vs baseline: 5.0893x; 1.0700x over previous
"""Point-Transformer attention block on 8 Trainium2 NeuronCores.

Shards the points axis N across 8 cores (all ops are pointwise in N).
Per core: channels on SBUF partitions, pixels on the free dim.  Pixels are
host-reordered k-major within each 256-point tile, so every (n,k) view
becomes a contiguous slice.  All matmuls use bf16 operands (1 cycle/row)
accumulating in fp32 PSUM.

Fusions vs the reference:
  - x3/d in one psum: stationary [w3; -w2] + [pw2; pw2] -> psum rows
    0:64 = x3_pre (+ptf), 64:128 = d = ptf - x2_pre; one scalar-engine
    activation adds [b3; 0] and materializes both halves as bf16.
  - ppfs host-packed [16, npix/2] (chunk pairs stacked on partitions) so
    one blockdiag(pw1) matmul computes two chunks of r at once.
  - x1 / xfs / cw1 folded:  h = relu( sum_k cw1_k^T @ d_k
                                      + (cw1s @ w1) @ x + cw1s @ (b1 - b2) )
  - softmax denominators applied at the END (U * bcast(1/sum)) so the
    reciprocal never blocks the tensor engine; exp -> 0/1-matmul group
    sums -> fast approx reciprocal -> f32r 0/1-matmul broadcast.
  - k-sum of q: pairwise adds on DVE then 8 accumulating identity matmuls
  - identity skip + bout folded into the final DVE scalar_tensor_tensor.
Emission is software-pipelined: S1(i) then an interleaved zone running
tile i-1's attention-apply against tile i's h/e computation, so neither
the tensor engine nor DVE ever drains.
"""

import numpy as np

B, CIN, N, K = 4, 64, 16384, 16
MID, OUT, PT, SHARE = 64, 128, 8, 8
G = MID // SHARE  # 8 softmax groups
NCORES = 8
NS = N // NCORES          # points per core per batch (2048)
P = 256                   # points per tile
PK = P * K                # pixels per tile (4096)
FD = 512                  # pixel chunk (one matmul's moving width)
NCH = PK // FD            # chunks per tile (8)


def _build_consts(w0, b0, w1, b1, w2, b2, w3, b3, pw1, pw2, cw1, cw2, cb2,
                  wout, bout):
    import ml_dtypes
    f32 = np.float32
    bf = ml_dtypes.bfloat16
    c = {}
    c["w0t"] = np.ascontiguousarray(w0.T).astype(bf)                # [64,128]
    c["w23t"] = np.ascontiguousarray(
        np.concatenate([w3, -w2], axis=0).T).astype(bf)             # [128,128]
    pw2t2 = np.ascontiguousarray(np.concatenate([pw2, pw2], axis=0).T)  # [64,128]
    z = np.zeros_like(pw2t2)
    c["pw2lo"] = np.concatenate([pw2t2, z], axis=0).astype(bf)      # [128,128]
    c["pw2hi"] = np.concatenate([z, pw2t2], axis=0).astype(bf)      # [128,128]
    pw1t = np.ascontiguousarray(pw1.T)                              # [8,64]
    pw1t2 = np.zeros((2 * PT, OUT), np.float32)
    pw1t2[0:PT, 0:MID] = pw1t
    pw1t2[PT:2 * PT, MID:OUT] = pw1t
    c["pw1t2"] = pw1t2.astype(bf)                                   # [16,128]
    cw1r = cw1.reshape(G, MID, K)
    cw1s = cw1r.sum(-1)                                             # [8,64]
    c["cat"] = np.ascontiguousarray((cw1s @ w1).T).astype(bf)       # [128,8]
    gkt = np.concatenate([cw1r[:, :, k].T for k in range(K)], axis=1)  # [64,128]
    c["gkt"] = np.concatenate(
        [np.zeros_like(gkt), gkt], axis=0).astype(bf)               # [128,128]
    c["hb"] = np.ascontiguousarray((cw1s @ (b1 - b2))[:, None], f32)  # [8,1]
    c["cw2t"] = np.ascontiguousarray(cw2.T).astype(bf)              # [8,128]
    c["cb2"] = np.ascontiguousarray(cb2[:, None], f32)              # [128,1]
    bsum = np.zeros((OUT, G), f32)
    for g in range(G):
        bsum[g * K:(g + 1) * K, g] = 1.0
    c["bsum"] = bsum.astype(bf)                                     # [128,8]
    bsum64 = np.zeros((G, MID), f32)
    for m in range(MID):
        bsum64[m % G, m] = 1.0
    c["bsum64"] = bsum64.astype(bf)                                 # [8,64]
    bksel = np.zeros((OUT, K * MID), f32)
    for k in range(K):
        for m in range(MID):
            bksel[(m % G) * K + k, k * MID + m] = 1.0
    c["bksel"] = bksel.astype(bf)                                   # [128,1024]
    c["woutt"] = np.ascontiguousarray(wout.T).astype(bf)            # [64,128]
    c["eye64"] = np.eye(MID, dtype=f32).astype(bf)                  # [64,64]
    c["b0"] = np.ascontiguousarray(b0[:, None], f32)                # [128,1]
    c["b3z"] = np.concatenate(
        [b3, np.zeros(MID, f32)])[:, None].astype(f32)              # [128,1]
    c["bout"] = np.ascontiguousarray(bout[:, None], f32)            # [128,1]
    return c


def _build_program(ns_pts):
    import concourse.bass as bass
    import concourse.tile as tile
    from concourse import mybir
    from contextlib import ExitStack

    f32 = mybir.dt.float32
    f32r = mybir.dt.float32r
    bf16 = mybir.dt.bfloat16
    AF = mybir.ActivationFunctionType
    ALU = mybir.AluOpType
    nt = ns_pts // P

    nc = bass.Bass()
    feats_d = nc.declare_dram_parameter("feats", [B, CIN, ns_pts * K], bf16, isOutput=False)
    ppfs_d = nc.declare_dram_parameter("ppfs", [B, 2 * PT, ns_pts * K // 2], bf16, isOutput=False)
    cshape = dict(
        w0t=([CIN, OUT], bf16), w23t=([OUT, OUT], bf16),
        pw2lo=([OUT, OUT], bf16), pw2hi=([OUT, OUT], bf16),
        pw1t2=([2 * PT, OUT], bf16), cat=([OUT, G], bf16), gkt=([OUT, K * G], bf16),
        hb=([G, 1], f32), cw2t=([G, OUT], bf16), cb2=([OUT, 1], f32),
        bsum=([OUT, G], bf16), bsum64=([G, MID], bf16), bksel=([OUT, K * MID], bf16),
        woutt=([MID, OUT], bf16), eye64=([MID, MID], bf16),
        b0=([OUT, 1], f32), b3z=([OUT, 1], f32), bout=([OUT, 1], f32),
    )
    cdram = {k: nc.declare_dram_parameter(k, v[0], v[1], isOutput=False)
             for k, v in cshape.items()}
    out_d = nc.declare_dram_parameter("out", [B, OUT, ns_pts], f32, isOutput=True)

    ntot = B * nt

    with tile.TileContext(nc) as tc, ExitStack() as ctx:
        consts = ctx.enter_context(tc.tile_pool(name="consts", bufs=1))
        ct = {k: consts.tile_from(v[:], name=k) for k, v in cdram.items()}

        io = ctx.enter_context(tc.tile_pool(name="io", bufs=2))
        sb = ctx.enter_context(tc.tile_pool(name="sb", bufs=2))
        small = ctx.enter_context(tc.tile_pool(name="small", bufs=2))
        pix = ctx.enter_context(tc.tile_pool(name="pix", bufs=2, space="PSUM"))
        ptp = ctx.enter_context(tc.tile_pool(name="ptp", bufs=2, space="PSUM"))
        wfp = ctx.enter_context(tc.tile_pool(name="wfp", bufs=2, space="PSUM"))

        st = {}  # per-tile live handles

        def emit_dma(i):
            b, t = divmod(i, nt)
            px = slice(t * PK, (t + 1) * PK)
            px2 = slice(t * PK // 2, (t + 1) * PK // 2)
            ft = io.tile([CIN, PK], bf16, tag="ft")
            nc.sync.dma_start(ft[:], feats_d[b, :, px])
            pf = io.tile([2 * PT, PK // 2], bf16, tag="pf")
            nc.sync.dma_start(pf[:], ppfs_d[b, :, px2])
            st[i] = dict(ft=ft, pf=pf)

        def emit_s1(i):
            s = st[i]
            ft, pf = s["ft"], s["pf"]
            # xn = relu(w0 @ feats + b0)            [128, PK] bf16
            xn = sb.tile([OUT, PK], bf16, tag="xn")
            for u in range(NCH // 2):
                ps = pix.tile([OUT, 2 * FD], f32, tag="pix")
                for h in range(2):
                    cs = slice((2 * u + h) * FD, (2 * u + h + 1) * FD)
                    nc.tensor.matmul(out=ps[:, h * FD:(h + 1) * FD],
                                     lhsT=ct["w0t"][:], rhs=ft[:, cs],
                                     start=True, stop=True)
                nc.scalar.activation(xn[:, 2 * u * FD:(2 * u + 2) * FD],
                                     ps[:], AF.Relu, bias=ct["b0"][:])
            # r = relu(pw1 @ ppfs), chunk c on rows 64*(c%2), cols (c//2)*FD
            r = sb.tile([OUT, PK // 2], bf16, tag="r")
            for u in range(NCH // 4):
                ps = pix.tile([OUT, 2 * FD], f32, tag="pix")
                for h in range(2):
                    js = slice((2 * u + h) * FD, (2 * u + h + 1) * FD)
                    nc.tensor.matmul(out=ps[:, h * FD:(h + 1) * FD],
                                     lhsT=ct["pw1t2"][:], rhs=pf[:, js],
                                     start=True, stop=True)
                eng = nc.scalar if u == 0 else None
                if eng is not None:
                    eng.activation(r[:, 2 * u * FD:(2 * u + 2) * FD],
                                   ps[:], AF.Relu)
                else:
                    nc.vector.tensor_scalar_max(
                        r[:, 2 * u * FD:(2 * u + 2) * FD], ps[:], 0.0)
            # psum = [w3@xn + pw2@r | -w2@xn + pw2@r]; +[b3|0] -> xd bf16
            xd = sb.tile([OUT, PK], bf16, tag="xd")
            for u in range(NCH // 2):
                ps = pix.tile([OUT, 2 * FD], f32, tag="pix")
                for h in range(2):
                    c = 2 * u + h
                    cs = slice(c * FD, (c + 1) * FD)
                    rcs = slice((c // 2) * FD, (c // 2 + 1) * FD)
                    hs = slice(h * FD, (h + 1) * FD)
                    nc.tensor.matmul(out=ps[:, hs], lhsT=ct["w23t"][:],
                                     rhs=xn[:, cs], start=True, stop=False)
                    nc.tensor.matmul(out=ps[:, hs],
                                     lhsT=ct["pw2lo" if c % 2 == 0 else "pw2hi"][:],
                                     rhs=r[:, rcs], start=False, stop=True)
                nc.scalar.activation(xd[:, 2 * u * FD:(2 * u + 2) * FD],
                                     ps[:], AF.Identity, bias=ct["b3z"][:])
            s["xn"], s["r"], s["xd"] = xn, r, xd

        def emit_zone(ip, ic):
            """Interleave tile ip's attention-apply (bksel/q/U/out) with
            tile ic's h/e/softmax-denominator computation."""
            sp = st.get(ip) if ip is not None else None
            sc = st.get(ic) if ic is not None else None
            if sc is not None:
                hps = ptp.tile([G, P], f32, tag="pt")
            if sp is not None:
                q = sb.tile([MID, PK], bf16, tag="q")
                q2 = sb.tile([MID, PK // 2], bf16, tag="q2")
                ups = ptp.tile([MID, P], f32, tag="pt")
            for j in range(K // 2):
                if sp is not None:
                    wf = wfp.tile([MID, 2 * P], f32, tag="wf")
                    for h in range(2):
                        k = 2 * j + h
                        nc.tensor.matmul(out=wf[:, h * P:(h + 1) * P],
                                         lhsT=ct["bksel"][:, k * MID:(k + 1) * MID],
                                         rhs=sp["e"][:], start=True, stop=True)
                if sc is not None:
                    for h in range(2):
                        k = 2 * j + h
                        nc.tensor.matmul(
                            out=hps[:], lhsT=ct["gkt"][:, k * G:(k + 1) * G],
                            rhs=sc["xd"][:, k * P:(k + 1) * P],
                            start=(k == 0), stop=False)
                if sp is not None:
                    # q_k = x3_k * wf_k (pairs); q2_j = q_2j + q_2j+1
                    nc.vector.tensor_mul(q[:, 2 * j * P:(2 * j + 2) * P],
                                         sp["xd"][0:MID, 2 * j * P:(2 * j + 2) * P],
                                         wf[:])
                    nc.vector.tensor_add(q2[:, j * P:(j + 1) * P],
                                         q[:, 2 * j * P:(2 * j + 1) * P],
                                         q[:, (2 * j + 1) * P:(2 * j + 2) * P])
                    nc.tensor.matmul(out=ups[:], lhsT=ct["eye64"][:],
                                     rhs=q2[:, j * P:(j + 1) * P],
                                     start=(j == 0), stop=(j == K // 2 - 1))
            if sc is not None:
                nc.tensor.matmul(out=hps[:], lhsT=ct["cat"][:],
                                 rhs=sc["xn"][:, 0:P], start=False, stop=True)
                h = small.tile([G, P], bf16, tag="h")
                nc.scalar.activation(h[:], hps[:], AF.Relu, bias=ct["hb"][:])
            if sp is not None:
                # o = relu(U * bcast(rs))
                rsb = ptp.tile([MID, P], f32, tag="pt")
                nc.tensor.matmul(out=rsb[:], lhsT=ct["bsum64"][:],
                                 rhs=sp["rs"][:], start=True, stop=True)
                # relu commutes with the positive softmax scale:
                # relu(U * rsb) == relu(U) * rsb
                un = small.tile([MID, P], bf16, tag="un")
                nc.scalar.activation(un[:], ups[:], AF.Relu)
                o = small.tile([MID, P], bf16, tag="o")
                nc.vector.tensor_mul(o[:], un[:], rsb[:])
            if sc is not None:
                # e = exp(cw2 @ h + cb2); sums = bsum @ e; rs = 1/sums
                eps = ptp.tile([OUT, P], f32, tag="pt")
                nc.tensor.matmul(out=eps[:], lhsT=ct["cw2t"][:], rhs=h[:],
                                 start=True, stop=True)
                e = small.tile([OUT, P], bf16, tag="e")
                nc.scalar.activation(e[:], eps[:], AF.Exp, bias=ct["cb2"][:])
                sps = ptp.tile([G, P], f32, tag="pt")
                nc.tensor.matmul(out=sps[:], lhsT=ct["bsum"][:], rhs=e[:],
                                 start=True, stop=True)
                rs = small.tile([G, P], bf16, tag="rs")
                with nc.allow_low_precision("softmax reciprocal in bf16"):
                    nc.vector.reciprocal(rs[:], sps[:])
                sc["e"], sc["rs"] = e, rs
            if sp is not None:
                # out = wout @ o + x + bout
                b, t = divmod(ip, nt)
                ops_ = ptp.tile([OUT, P], f32, tag="pt")
                nc.tensor.matmul(out=ops_[:], lhsT=ct["woutt"][:], rhs=o[:],
                                 start=True, stop=True)
                res = small.tile([OUT, P], f32, tag="res")
                nc.vector.scalar_tensor_tensor(
                    res[:], ops_[:], ct["bout"][:], sp["xn"][:, 0:P],
                    op0=ALU.add, op1=ALU.add)
                nc.sync.dma_start(out_d[b, :, t * P:(t + 1) * P], res[:])
                del st[ip]

        emit_dma(0)
        for i in range(ntot):
            if i + 1 < ntot:
                emit_dma(i + 1)
            emit_s1(i)
            emit_zone(i - 1 if i >= 1 else None, i)
        emit_zone(ntot - 1, None)

    return nc


def _legalize_waits(nc):
    """This toolchain's walrus rejects >1 sync-wait per instruction; hoist
    extra waits onto same-engine event-semaphore instructions just before."""
    from concourse import mybir

    n_split = 0
    for fn in nc.m.functions:
        for bb in fn.blocks:
            insts = bb.instructions
            new_list = []
            for inst in insts:
                si = inst.sync_info
                if si is not None and si.on_wait is not None and len(si.on_wait) > 1:
                    waits = list(si.on_wait)
                    for j, w in enumerate(waits[:-1]):
                        ev = mybir.InstEventSemaphore(
                            name=f"{inst.name}-lw{j}", ins=[], outs=[])
                        ev.engine = inst.engine
                        ev.sync_info = mybir.SyncInfo(on_wait=[w], on_update=[])
                        new_list.append(ev)
                        n_split += 1
                    inst.sync_info = mybir.SyncInfo(
                        on_wait=[waits[-1]], on_update=list(si.on_update))
                new_list.append(inst)
            if len(new_list) != len(insts):
                insts[:] = new_list
    return n_split


LAST_RESULTS = None


def kernel(sm_feats, sm_ppfs, w0, b0, w1, b1, w2, b2, w3, b3,
           pw1, pw2, cw1, cw2, cb2, wout, bout):
    global LAST_RESULTS
    import ml_dtypes
    from concourse.bass_utils import run_bass_kernel_spmd

    bf = ml_dtypes.bfloat16
    consts = _build_consts(w0, b0, w1, b1, w2, b2, w3, b3, pw1, pw2,
                           cw1, cw2, cb2, wout, bout)
    nc = _build_program(NS)
    _legalize_waits(nc)

    feats_bf = np.ascontiguousarray(sm_feats).astype(bf)
    ppfs_bf = np.ascontiguousarray(sm_ppfs).astype(bf)
    npix = NS * K

    in_maps = []
    for i in range(NCORES):
        sl = slice(i * NS, (i + 1) * NS)
        m = dict(consts)
        # k-major pixel order within each 256-point tile
        f = feats_bf[:, :, sl, :].reshape(B, CIN, NS // P, P, K)
        m["feats"] = np.ascontiguousarray(
            f.transpose(0, 1, 2, 4, 3)).reshape(B, CIN, npix)
        p = ppfs_bf[:, :, sl, :].reshape(B, PT, NS // P, P, K)
        pp = np.ascontiguousarray(
            p.transpose(0, 1, 2, 4, 3)).reshape(B, PT, npix)
        # pack chunk pairs on partitions: [b, h*8+ch, j*512+s] = pp[b, ch, (2j+h)*512+s]
        v = pp.reshape(B, PT, npix // 1024, 2, FD)
        m["ppfs"] = np.ascontiguousarray(
            v.transpose(0, 3, 1, 2, 4)).reshape(B, 2 * PT, npix // 2)
        in_maps.append(m)

    res = run_bass_kernel_spmd(nc, in_maps, list(range(NCORES)))
    LAST_RESULTS = res
    shards = [res.results[i]["out"] for i in range(NCORES)]
    return np.concatenate(shards, axis=2)


# revision 13
# speedup vs baseline: 5.4904x; 1.0788x over previous
"""Point-Transformer attention block on 8 Trainium2 NeuronCores.

Shards the points axis N across 8 cores (all ops are pointwise in N).
Per core: channels on SBUF partitions, pixels on the free dim.  Pixels are
host-reordered k-major within each 256-point tile, so every (n,k) view
becomes a contiguous slice.  All matmuls use bf16 operands (1 cycle/row)
accumulating in fp32 PSUM.

Fusions vs the reference:
  - x3/d in one psum: stationary [w3; -w2] + [pw2; pw2] -> psum rows
    0:64 = x3_pre (+ptf), 64:128 = d = ptf - x2_pre; one scalar-engine
    activation adds [b3; 0] and materializes both halves as bf16.
  - ppfs host-packed [16, npix/2] (chunk pairs stacked on partitions) so
    one blockdiag(pw1) matmul computes two chunks of r at once.
  - x1 / xfs / cw1 folded:  h = relu( sum_k cw1_k^T @ d_k
                                      + (cw1s @ w1) @ x + cw1s @ (b1 - b2) )
  - softmax denominators applied at the END (U * bcast(1/sum)) so the
    reciprocal never blocks the tensor engine; exp -> 0/1-matmul group
    sums -> fast approx reciprocal -> f32r 0/1-matmul broadcast.
  - k-sum of q: pairwise adds on DVE then 8 accumulating identity matmuls
  - identity skip + bout folded into the final DVE scalar_tensor_tensor.
Emission is software-pipelined: S1(i) then an interleaved zone running
tile i-1's attention-apply against tile i's h/e computation, so neither
the tensor engine nor DVE ever drains.
"""

import numpy as np

B, CIN, N, K = 4, 64, 16384, 16
MID, OUT, PT, SHARE = 64, 128, 8, 8
G = MID // SHARE  # 8 softmax groups
NCORES = 8
NS = N // NCORES          # points per core per batch (2048)
P = 256                   # points per tile
PK = P * K                # pixels per tile (4096)
FD = 512                  # pixel chunk (one matmul's moving width)
NCH = PK // FD            # chunks per tile (8)


def _build_consts(w0, b0, w1, b1, w2, b2, w3, b3, pw1, pw2, cw1, cw2, cb2,
                  wout, bout):
    import ml_dtypes
    f32 = np.float32
    bf = ml_dtypes.bfloat16
    c = {}
    c["w0t"] = np.ascontiguousarray(w0.T).astype(bf)                # [64,128]
    c["w23t"] = np.ascontiguousarray(
        np.concatenate([w3, -w2], axis=0).T).astype(bf)             # [128,128]
    pw2t2 = np.ascontiguousarray(np.concatenate([pw2, pw2], axis=0).T)  # [64,128]
    z = np.zeros_like(pw2t2)
    c["pw2lo"] = np.concatenate([pw2t2, z], axis=0).astype(bf)      # [128,128]
    c["pw2hi"] = np.concatenate([z, pw2t2], axis=0).astype(bf)      # [128,128]
    pw1t = np.ascontiguousarray(pw1.T)                              # [8,64]
    pw1t2 = np.zeros((2 * PT, OUT), np.float32)
    pw1t2[0:PT, 0:MID] = pw1t
    pw1t2[PT:2 * PT, MID:OUT] = pw1t
    c["pw1t2"] = pw1t2.astype(bf)                                   # [16,128]
    cw1r = cw1.reshape(G, MID, K)
    cw1s = cw1r.sum(-1)                                             # [8,64]
    c["cat"] = np.ascontiguousarray((cw1s @ w1).T).astype(bf)       # [128,8]
    gkt = np.concatenate([cw1r[:, :, k].T for k in range(K)], axis=1)  # [64,128]
    c["gkt"] = np.concatenate(
        [np.zeros_like(gkt), gkt], axis=0).astype(bf)               # [128,128]
    c["hb"] = np.ascontiguousarray((cw1s @ (b1 - b2))[:, None], f32)  # [8,1]
    c["cw2t"] = np.ascontiguousarray(cw2.T).astype(bf)              # [8,128]
    c["cb2"] = np.ascontiguousarray(cb2[:, None], f32)              # [128,1]
    bsum = np.zeros((OUT, G), f32)
    for g in range(G):
        bsum[g * K:(g + 1) * K, g] = 1.0
    c["bsum"] = bsum.astype(bf)                                     # [128,8]
    bsum64 = np.zeros((G, MID), f32)
    for m in range(MID):
        bsum64[m % G, m] = 1.0
    c["bsum64"] = bsum64.astype(bf)                                 # [8,64]
    bksel = np.zeros((OUT, K * MID), f32)
    for k in range(K):
        for m in range(MID):
            bksel[(m % G) * K + k, k * MID + m] = 1.0
    c["bksel"] = bksel.astype(bf)                                   # [128,1024]
    c["woutt"] = np.ascontiguousarray(wout.T).astype(bf)            # [64,128]
    c["eye64"] = np.eye(MID, dtype=f32).astype(bf)                  # [64,64]
    c["b0"] = np.ascontiguousarray(b0[:, None], f32)                # [128,1]
    c["b3z"] = np.concatenate(
        [b3, np.zeros(MID, f32)])[:, None].astype(f32)              # [128,1]
    c["bout"] = np.ascontiguousarray(bout[:, None], f32)            # [128,1]
    return c


def _build_program(ns_pts):
    import concourse.bass as bass
    import concourse.tile as tile
    from concourse import mybir
    from contextlib import ExitStack

    f32 = mybir.dt.float32
    f32r = mybir.dt.float32r
    bf16 = mybir.dt.bfloat16
    AF = mybir.ActivationFunctionType
    ALU = mybir.AluOpType
    nt = ns_pts // P

    nc = bass.Bass()
    feats_d = nc.declare_dram_parameter("feats", [B, CIN, ns_pts * K], bf16, isOutput=False)
    ppfs_d = nc.declare_dram_parameter("ppfs", [B, 2 * PT, ns_pts * K // 2], bf16, isOutput=False)
    cshape = dict(
        w0t=([CIN, OUT], bf16), w23t=([OUT, OUT], bf16),
        pw2lo=([OUT, OUT], bf16), pw2hi=([OUT, OUT], bf16),
        pw1t2=([2 * PT, OUT], bf16), cat=([OUT, G], bf16), gkt=([OUT, K * G], bf16),
        hb=([G, 1], f32), cw2t=([G, OUT], bf16), cb2=([OUT, 1], f32),
        bsum=([OUT, G], bf16), bsum64=([G, MID], bf16), bksel=([OUT, K * MID], bf16),
        woutt=([MID, OUT], bf16), eye64=([MID, MID], bf16),
        b0=([OUT, 1], f32), b3z=([OUT, 1], f32), bout=([OUT, 1], f32),
    )
    cdram = {k: nc.declare_dram_parameter(k, v[0], v[1], isOutput=False)
             for k, v in cshape.items()}
    out_d = nc.declare_dram_parameter("out", [B, OUT, ns_pts], f32, isOutput=True)

    ntot = B * nt

    with tile.TileContext(nc) as tc, ExitStack() as ctx:
        consts = ctx.enter_context(tc.tile_pool(name="consts", bufs=1))
        ct = {k: consts.tile_from(v[:], name=k) for k, v in cdram.items()}

        io = ctx.enter_context(tc.tile_pool(name="io", bufs=2))
        sb = ctx.enter_context(tc.tile_pool(name="sb", bufs=2))
        small = ctx.enter_context(tc.tile_pool(name="small", bufs=2))
        pix = ctx.enter_context(tc.tile_pool(name="pix", bufs=2, space="PSUM"))
        ptp = ctx.enter_context(tc.tile_pool(name="ptp", bufs=2, space="PSUM"))
        wfp = ctx.enter_context(tc.tile_pool(name="wfp", bufs=2, space="PSUM"))

        st = {}  # per-tile live handles

        def emit_dma(i):
            b, t = divmod(i, nt)
            px = slice(t * PK, (t + 1) * PK)
            px2 = slice(t * PK // 2, (t + 1) * PK // 2)
            ft = io.tile([CIN, PK], bf16, tag="ft")
            nc.sync.dma_start(ft[:], feats_d[b, :, px])
            pf = io.tile([2 * PT, PK // 2], bf16, tag="pf")
            nc.sync.dma_start(pf[:], ppfs_d[b, :, px2])
            st[i] = dict(ft=ft, pf=pf)

        def emit_s1a(i):
            s = st[i]
            ft, pf = s["ft"], s["pf"]
            # xn = relu(w0 @ feats + b0)            [128, PK] bf16
            xn = sb.tile([OUT, PK], bf16, tag="xn")
            for u in range(NCH // 2):
                ps = pix.tile([OUT, 2 * FD], f32, tag="pix")
                for h in range(2):
                    cs = slice((2 * u + h) * FD, (2 * u + h + 1) * FD)
                    nc.tensor.matmul(out=ps[:, h * FD:(h + 1) * FD],
                                     lhsT=ct["w0t"][:], rhs=ft[:, cs],
                                     start=True, stop=True)
                nc.scalar.activation(xn[:, 2 * u * FD:(2 * u + 2) * FD],
                                     ps[:], AF.Relu, bias=ct["b0"][:])
            # r = relu(pw1 @ ppfs), chunk c on rows 64*(c%2), cols (c//2)*FD
            r = sb.tile([OUT, PK // 2], bf16, tag="r")
            for u in range(NCH // 4):
                ps = pix.tile([OUT, 2 * FD], f32, tag="pix")
                for h in range(2):
                    js = slice((2 * u + h) * FD, (2 * u + h + 1) * FD)
                    nc.tensor.matmul(out=ps[:, h * FD:(h + 1) * FD],
                                     lhsT=ct["pw1t2"][:], rhs=pf[:, js],
                                     start=True, stop=True)
                if u == 0:
                    nc.scalar.activation(r[:, 0:2 * FD], ps[:], AF.Relu)
                else:
                    nc.vector.tensor_scalar_max(
                        r[:, 2 * FD:4 * FD], ps[:], 0.0)
            s["xn"], s["r"] = xn, r

        def emit_s1b(i):
            s = st[i]
            xn, r = s["xn"], s["r"]
            # psum = [w3@xn + pw2@r | -w2@xn + pw2@r]; +[b3|0] -> xd bf16
            xd = sb.tile([OUT, PK], bf16, tag="xd")
            for u in range(NCH // 2):
                ps = pix.tile([OUT, 2 * FD], f32, tag="pix")
                for h in range(2):
                    c = 2 * u + h
                    cs = slice(c * FD, (c + 1) * FD)
                    rcs = slice((c // 2) * FD, (c // 2 + 1) * FD)
                    hs = slice(h * FD, (h + 1) * FD)
                    nc.tensor.matmul(out=ps[:, hs], lhsT=ct["w23t"][:],
                                     rhs=xn[:, cs], start=True, stop=False)
                    nc.tensor.matmul(out=ps[:, hs],
                                     lhsT=ct["pw2lo" if c % 2 == 0 else "pw2hi"][:],
                                     rhs=r[:, rcs], start=False, stop=True)
                nc.scalar.activation(xd[:, 2 * u * FD:(2 * u + 2) * FD],
                                     ps[:], AF.Identity, bias=ct["b3z"][:])
            s["xd"] = xd

        def emit_qhalf(ip, jlo, jhi):
            # bksel pairs [jlo, jhi) for tile ip, with DVE q-mul/q-add chasers
            sp = st[ip]
            q, q2 = sp["q"], sp["q2"]
            for j in range(jlo, jhi):
                wf = wfp.tile([MID, 2 * P], f32, tag="wf")
                for h in range(2):
                    k = 2 * j + h
                    nc.tensor.matmul(out=wf[:, h * P:(h + 1) * P],
                                     lhsT=ct["bksel"][:, k * MID:(k + 1) * MID],
                                     rhs=sp["e"][:], start=True, stop=True)
                nc.vector.tensor_mul(q[:, 2 * j * P:(2 * j + 2) * P],
                                     sp["xd"][0:MID, 2 * j * P:(2 * j + 2) * P],
                                     wf[:])
                nc.vector.tensor_add(q2[:, j * P:(j + 1) * P],
                                     q[:, 2 * j * P:(2 * j + 1) * P],
                                     q[:, (2 * j + 1) * P:(2 * j + 2) * P])

        def emit_zone_b(ip):
            sp = st[ip]
            sp["q"] = sb.tile([MID, PK], bf16, tag="q", name="q")
            sp["q2"] = sb.tile([MID, PK // 2], bf16, tag="q2", name="q2")
            emit_qhalf(ip, 0, K // 4)

        def emit_zone_d(ip):
            sp = st[ip]
            emit_qhalf(ip, K // 4, K // 2)
            q2 = sp["q2"]
            ups = ptp.tile([MID, P], f32, tag="pt")
            for j in range(K // 2):
                nc.tensor.matmul(out=ups[:], lhsT=ct["eye64"][:],
                                 rhs=q2[:, j * P:(j + 1) * P],
                                 start=(j == 0), stop=(j == K // 2 - 1))
            # o = relu(U) * bcast(rs)   (relu commutes with positive scale)
            rsb = ptp.tile([MID, P], f32, tag="pt")
            nc.tensor.matmul(out=rsb[:], lhsT=ct["bsum64"][:],
                             rhs=sp["rs"][:], start=True, stop=True)
            un = small.tile([MID, P], bf16, tag="un")
            nc.scalar.activation(un[:], ups[:], AF.Relu)
            o = small.tile([MID, P], bf16, tag="o")
            nc.vector.tensor_mul(o[:], un[:], rsb[:])
            # out = wout @ o + x + bout
            b, t = divmod(ip, nt)
            ops_ = ptp.tile([OUT, P], f32, tag="pt")
            nc.tensor.matmul(out=ops_[:], lhsT=ct["woutt"][:], rhs=o[:],
                             start=True, stop=True)
            res = small.tile([OUT, P], f32, tag="res")
            nc.vector.scalar_tensor_tensor(
                res[:], ops_[:], ct["bout"][:], sp["xn"][:, 0:P],
                op0=ALU.add, op1=ALU.add)
            nc.sync.dma_start(out_d[b, :, t * P:(t + 1) * P], res[:])
            del st[ip]

        def emit_zone_e(ic):
            sc = st[ic]
            # h = relu(sum_k cw1_k^T @ d_k + Ca @ x + hb)   [8, P]
            hps = ptp.tile([G, P], f32, tag="pt")
            for k in range(K):
                nc.tensor.matmul(out=hps[:], lhsT=ct["gkt"][:, k * G:(k + 1) * G],
                                 rhs=sc["xd"][:, k * P:(k + 1) * P],
                                 start=(k == 0), stop=False)
            nc.tensor.matmul(out=hps[:], lhsT=ct["cat"][:],
                             rhs=sc["xn"][:, 0:P], start=False, stop=True)
            h = small.tile([G, P], bf16, tag="h")
            nc.scalar.activation(h[:], hps[:], AF.Relu, bias=ct["hb"][:])
            # e = exp(cw2 @ h + cb2); sums = bsum @ e; rs = 1/sums
            eps = ptp.tile([OUT, P], f32, tag="pt")
            nc.tensor.matmul(out=eps[:], lhsT=ct["cw2t"][:], rhs=h[:],
                             start=True, stop=True)
            e = small.tile([OUT, P], bf16, tag="e")
            nc.scalar.activation(e[:], eps[:], AF.Exp, bias=ct["cb2"][:])
            sps = ptp.tile([G, P], f32, tag="pt")
            nc.tensor.matmul(out=sps[:], lhsT=ct["bsum"][:], rhs=e[:],
                             start=True, stop=True)
            rs = small.tile([G, P], bf16, tag="rs")
            with nc.allow_low_precision("softmax reciprocal in bf16"):
                nc.vector.reciprocal(rs[:], sps[:])
            sc["e"], sc["rs"] = e, rs

        emit_dma(0)
        for i in range(ntot):
            if i + 1 < ntot:
                emit_dma(i + 1)
            emit_s1a(i)
            if i >= 1:
                emit_zone_b(i - 1)
            emit_s1b(i)
            if i >= 1:
                emit_zone_d(i - 1)
            emit_zone_e(i)
        emit_zone_b(ntot - 1)
        emit_zone_d(ntot - 1)

    return nc


def _legalize_waits(nc):
    """This toolchain's walrus rejects >1 sync-wait per instruction; hoist
    extra waits onto same-engine event-semaphore instructions just before."""
    from concourse import mybir

    n_split = 0
    for fn in nc.m.functions:
        for bb in fn.blocks:
            insts = bb.instructions
            new_list = []
            for inst in insts:
                si = inst.sync_info
                if si is not None and si.on_wait is not None and len(si.on_wait) > 1:
                    waits = list(si.on_wait)
                    for j, w in enumerate(waits[:-1]):
                        ev = mybir.InstEventSemaphore(
                            name=f"{inst.name}-lw{j}", ins=[], outs=[])
                        ev.engine = inst.engine
                        ev.sync_info = mybir.SyncInfo(on_wait=[w], on_update=[])
                        new_list.append(ev)
                        n_split += 1
                    inst.sync_info = mybir.SyncInfo(
                        on_wait=[waits[-1]], on_update=list(si.on_update))
                new_list.append(inst)
            if len(new_list) != len(insts):
                insts[:] = new_list
    return n_split


LAST_RESULTS = None


def kernel(sm_feats, sm_ppfs, w0, b0, w1, b1, w2, b2, w3, b3,
           pw1, pw2, cw1, cw2, cb2, wout, bout):
    global LAST_RESULTS
    import ml_dtypes
    from concourse.bass_utils import run_bass_kernel_spmd

    bf = ml_dtypes.bfloat16
    consts = _build_consts(w0, b0, w1, b1, w2, b2, w3, b3, pw1, pw2,
                           cw1, cw2, cb2, wout, bout)
    nc = _build_program(NS)
    _legalize_waits(nc)

    feats_bf = np.ascontiguousarray(sm_feats).astype(bf)
    ppfs_bf = np.ascontiguousarray(sm_ppfs).astype(bf)
    npix = NS * K

    in_maps = []
    for i in range(NCORES):
        sl = slice(i * NS, (i + 1) * NS)
        m = dict(consts)
        # k-major pixel order within each 256-point tile
        f = feats_bf[:, :, sl, :].reshape(B, CIN, NS // P, P, K)
        m["feats"] = np.ascontiguousarray(
            f.transpose(0, 1, 2, 4, 3)).reshape(B, CIN, npix)
        p = ppfs_bf[:, :, sl, :].reshape(B, PT, NS // P, P, K)
        pp = np.ascontiguousarray(
            p.transpose(0, 1, 2, 4, 3)).reshape(B, PT, npix)
        # pack chunk pairs on partitions: [b, h*8+ch, j*512+s] = pp[b, ch, (2j+h)*512+s]
        v = pp.reshape(B, PT, npix // 1024, 2, FD)
        m["ppfs"] = np.ascontiguousarray(
            v.transpose(0, 3, 1, 2, 4)).reshape(B, 2 * PT, npix // 2)
        in_maps.append(m)

    res = run_bass_kernel_spmd(nc, in_maps, list(range(NCORES)))
    LAST_RESULTS = res
    shards = [res.results[i]["out"] for i in range(NCORES)]
    return np.concatenate(shards, axis=2)
